# revision 2
# baseline (speedup 1.0000x reference)
"""GTN (graph transformer network) forward on 8 Trainium2 cores.

Math (identical to the reference, right-associated):
  A_t = dense adjacency per edge type; A_i[c] = softmax(w_i)[c] . A
  H1 = A1@A2, H = rownorm(H1); out rows = rownorm(H@A3) @ XW.
  rownorm commutes through the left matmul, so
      rownorm(rownorm(A1@A2) @ A3) == rownorm(A1@A2@A3)
  and with XW1 = [X@gcn_w | 1], the chain right-associates:
      Z = A1 @ (A2 @ (A3 @ XW1))        (~26 GFLOP instead of ~550)
  Z[:,128] = rowsum(A1@A2@A3) gives the normalizer. Only the rows in
  target_x are ever read, so the last stage uses A1[target_x, :].

Sharding: channel c = core//4, row-quarter q = core%4.
  stage A: T3[q] = A3[c][rows_q,:] @ XW1     -> AllGather over channel group
  stage B: T2[q] = A2[c][rows_q,:] @ T3      -> AllGather over channel group
  stage C: Z[q]  = A1[c][targets_q,:] @ T2   -> output [256, 132] f32
All matmul inputs bf16 (rel err ~1e-3, gate 2e-2), PSUM accumulation f32.
Host ships the big matrices pre-transposed (built transposed at scatter
time, no transpose cost) so matmul lhsT tiles are natural slices.
"""

import os
import time
import numpy as np
from contextlib import ExitStack

NUM_EDGE = 5
C = 2
N = 4096
W_IN = 512
W_OUT = 128
NCORES = 8
P = 128
NK = N // P              # 32 contraction chunks
RQ = N // 4              # 1024 rows per core in stages A/B
MBA = RQ // P            # 8 output row-blocks, stages A/B
NT = 1024                # n_target
TQ = NT // 4             # 256 target rows per core in stage C
MBC = TQ // P            # 2 output row-blocks, stage C
DOUT = W_OUT + 4         # 132: XW cols + ones col + 3 zero pad
GROUPS = [[0, 1, 2, 3], [4, 5, 6, 7]]

_NC_CACHE = {}
LAST_EXEC_NS = None


def _build_nc():
    import concourse.tile as tile
    from concourse import bacc, mybir

    nc = bacc.Bacc("TRN2", target_bir_lowering=False, debug=False,
                   num_devices=NCORES)
    f32 = mybir.dt.float32
    bf16 = mybir.dt.bfloat16

    a3t = nc.dram_tensor("a3t", [N, RQ], bf16, kind="ExternalInput").ap()
    a2t = nc.dram_tensor("a2t", [N, RQ], bf16, kind="ExternalInput").ap()
    a1t = nc.dram_tensor("a1t", [N, TQ], bf16, kind="ExternalInput").ap()
    xw = nc.dram_tensor("xw", [N, DOUT], bf16, kind="ExternalInput").ap()
    z = nc.dram_tensor("z", [TQ, DOUT], f32, kind="ExternalOutput").ap()

    with tile.TileContext(nc) as tc, ExitStack() as ctx:
        sbp = ctx.enter_context(tc.tile_pool(name="sbp", bufs=1))
        dram = ctx.enter_context(tc.tile_pool(name="dram", bufs=1, space="DRAM"))
        ps = ctx.enter_context(tc.tile_pool(name="ps", bufs=4, space="PSUM"))

        # SBUF residents. layout [p, (k m)]: partition p = contraction index
        # within chunk k; column m = output row within the quarter.
        xw_sb = sbp.tile([P, NK * DOUT], bf16)
        a3_sb = sbp.tile([P, NK * RQ], bf16)
        a2_sb = sbp.tile([P, NK * RQ], bf16)
        a1_sb = sbp.tile([P, NK * TQ], bf16)
        t3_sb = sbp.tile([P, NK * DOUT], bf16)
        t2_sb = sbp.tile([P, NK * DOUT], bf16)
        t3p_sb = sbp.tile([P, MBA * DOUT], bf16)
        t2p_sb = sbp.tile([P, MBA * DOUT], bf16)
        z_sb = sbp.tile([P, MBC * DOUT], f32)

        # DRAM bounce buffers for the collectives (I/O tensors not allowed)
        t3p_d = dram.tile([RQ, DOUT], bf16)
        t3f_d = dram.tile([N, DOUT], bf16)
        t2p_d = dram.tile([RQ, DOUT], bf16)
        t2f_d = dram.tile([N, DOUT], bf16)

        nc.gpsimd.dma_start(
            xw_sb[:].rearrange("p (k d) -> p k d", k=NK),
            xw.rearrange("(k p) d -> p k d", p=P))
        nc.gpsimd.dma_start(
            a3_sb[:].rearrange("p (k m) -> p k m", k=NK),
            a3t.rearrange("(k p) m -> p k m", p=P))
        nc.sync.dma_start(
            a2_sb[:].rearrange("p (k m) -> p k m", k=NK),
            a2t.rearrange("(k p) m -> p k m", p=P))
        nc.sync.dma_start(
            a1_sb[:].rearrange("p (k m) -> p k m", k=NK),
            a1t.rearrange("(k p) m -> p k m", p=P))

        def stage(lhs_sb, rows, rhs_sb, out_sb, nmb):
            # out[m*P+p, :] = sum_k lhsT[kchunk, mblock]^T @ rhs[kchunk, :]
            for m in range(nmb):
                acc = ps.tile([P, DOUT], f32, tag="acc")
                for k in range(NK):
                    nc.tensor.matmul(
                        acc[:],
                        lhs_sb[:, k * rows + m * P:k * rows + (m + 1) * P],
                        rhs_sb[:, k * DOUT:(k + 1) * DOUT],
                        start=(k == 0), stop=(k == NK - 1))
                nc.vector.tensor_copy(out_sb[:, m * DOUT:(m + 1) * DOUT], acc[:])

        # stage A: T3 part = A3[rows_q,:] @ XW1
        stage(a3_sb, RQ, xw_sb, t3p_sb, MBA)
        nc.sync.dma_start(
            t3p_d[:].rearrange("(m p) d -> p m d", p=P),
            t3p_sb[:].rearrange("p (m d) -> p m d", m=MBA))
        nc.gpsimd.collective_compute(
            "AllGather", mybir.AluOpType.bypass, replica_groups=GROUPS,
            ins=[t3p_d.opt()], outs=[t3f_d.opt()])
        nc.gpsimd.dma_start(
            t3_sb[:].rearrange("p (k d) -> p k d", k=NK),
            t3f_d[:].rearrange("(k p) d -> p k d", p=P))

        # stage B: T2 part = A2[rows_q,:] @ T3
        stage(a2_sb, RQ, t3_sb, t2p_sb, MBA)
        nc.sync.dma_start(
            t2p_d[:].rearrange("(m p) d -> p m d", p=P),
            t2p_sb[:].rearrange("p (m d) -> p m d", m=MBA))
        nc.gpsimd.collective_compute(
            "AllGather", mybir.AluOpType.bypass, replica_groups=GROUPS,
            ins=[t2p_d.opt()], outs=[t2f_d.opt()])
        nc.gpsimd.dma_start(
            t2_sb[:].rearrange("p (k d) -> p k d", k=NK),
            t2f_d[:].rearrange("(k p) d -> p k d", p=P))

        # stage C: Z part = A1[targets_q,:] @ T2
        stage(a1_sb, TQ, t2_sb, z_sb, MBC)
        nc.sync.dma_start(
            z.rearrange("(m p) d -> p m d", p=P),
            z_sb[:].rearrange("p (m d) -> p m d", m=MBC))

    nc.compile()
    return nc


def _get_nc():
    if "nc" not in _NC_CACHE:
        _NC_CACHE["nc"] = _build_nc()
    return _NC_CACHE["nc"]


def _softmax_rows(w):
    w = np.asarray(w, np.float32)
    e = np.exp(w - w.max(axis=1, keepdims=True))
    return (e / e.sum(axis=1, keepdims=True)).astype(np.float32)


def _install_ntff_hook():
    """Recreate antenv.axon_hooks if the image lacks it (profiling only)."""
    import sys
    import types
    try:
        from antenv.axon_hooks import get_axon_ntff_profile_hook  # noqa: F401
        return
    except ImportError:
        pass
    try:
        from trn_agent_boot.trn_boot import _ntff_profile_via_ctypes
        import antenv
        mod = types.ModuleType("antenv.axon_hooks")
        state = {"h": None}
        mod.set_axon_ntff_profile_hook = lambda h: state.__setitem__("h", h)
        mod.get_axon_ntff_profile_hook = lambda: state["h"]
        sys.modules["antenv.axon_hooks"] = mod
        antenv.axon_hooks = mod
        mod.set_axon_ntff_profile_hook(
            _ntff_profile_via_ctypes("/opt/axon/libaxon_pjrt.so"))
    except Exception:
        pass


def _install_neff_cache():
    """Disk-cache the BIR->NEFF compile (keyed by exact BIR bytes)."""
    try:
        import hashlib
        import shutil
        import concourse.bass2jax as b2j
        if getattr(b2j, "_gtn_neff_cache", None):
            return
        orig = b2j.compile_bir_kernel
        root = "/var/tmp/gtn_neff_cache"

        def cached(bir_json, tmpdir, neff_name="file.neff"):
            d = None
            try:
                key = hashlib.sha256(bir_json).hexdigest()[:32]
                d = os.path.join(root, key)
                src = os.path.join(d, "cached.neff")
                if os.path.exists(src):
                    dst = os.path.join(tmpdir, neff_name)
                    shutil.copy(src, dst)
                    return dst
            except Exception:
                pass
            out = orig(bir_json, tmpdir, neff_name)
            try:
                if d is not None:
                    os.makedirs(d, exist_ok=True)
                    tmp = src + f".tmp.{os.getpid()}"
                    shutil.copy(out, tmp)
                    os.replace(tmp, src)
            except Exception:
                pass
            return out

        b2j.compile_bir_kernel = cached
        b2j._gtn_neff_cache = True
    except Exception:
        pass


def kernel(edge_index, edge_value, X, target_x, w_l0_c1, w_l0_c2, w_l1_c1,
           gcn_w, gcn_b, lin_w, lin_b):
    global LAST_EXEC_NS
    import ml_dtypes
    from concourse.bass_utils import run_bass_kernel_spmd
    bf16 = ml_dtypes.bfloat16

    src = np.asarray(edge_index[:, 0], np.int64)
    dst = np.asarray(edge_index[:, 1], np.int64)
    val = np.asarray(edge_value, np.float32)
    tx = np.asarray(target_x, np.int64)

    # transposed adjacency stack AT[t] = A_t^T (scatter with swapped indices;
    # duplicate edges accumulate)
    AT = np.zeros((NUM_EDGE, N * N), np.float32)
    for t in range(NUM_EDGE):
        np.add.at(AT[t], dst[t] * N + src[t], val[t])

    f1 = _softmax_rows(w_l0_c1)
    f2 = _softmax_rows(w_l0_c2)
    f3 = _softmax_rows(w_l1_c1)

    # full transposed combos for layers whose every row participates
    A2T = (f2 @ AT).reshape(C, N, N)
    A3T = (f3 @ AT).reshape(C, N, N)
    # stage C only ever reads the target rows of A1 == target cols of A1T
    G = AT.reshape(NUM_EDGE, N, N)[:, :, tx].reshape(NUM_EDGE, -1)
    A1G = (f1 @ G).reshape(C, N, NT)
    AT = None
    G = None

    XW = np.asarray(X, np.float32) @ np.asarray(gcn_w, np.float32)
    xw1 = np.concatenate([XW, np.ones((N, 1), np.float32),
                          np.zeros((N, 3), np.float32)], axis=1)
    xw1 = xw1.astype(bf16)

    in_maps = []
    for ci in range(NCORES):
        c, q = divmod(ci, 4)
        in_maps.append({
            "a3t": A3T[c][:, q * RQ:(q + 1) * RQ].astype(bf16),
            "a2t": A2T[c][:, q * RQ:(q + 1) * RQ].astype(bf16),
            "a1t": A1G[c][:, q * TQ:(q + 1) * TQ].astype(bf16),
            "xw": xw1,
        })

    _install_neff_cache()
    nc = _get_nc()
    trace = bool(int(os.environ.get("GTN_TRACE", "1")))
    if trace:
        _install_ntff_hook()

    t0 = time.time()
    try:
        res = run_bass_kernel_spmd(nc, in_maps, list(range(NCORES)),
                                   trace=trace)
    except Exception:
        if not trace:
            raise
        t0 = time.time()
        res = run_bass_kernel_spmd(nc, in_maps, list(range(NCORES)),
                                   trace=False)
    wall_ns = int((time.time() - t0) * 1e9)
    LAST_EXEC_NS = res.exec_time_ns if res.exec_time_ns else wall_ns

    gcn_b = np.asarray(gcn_b, np.float32)
    outs = []
    for c in range(C):
        Zt = np.concatenate([res.results[4 * c + q]["z"] for q in range(4)])
        s = Zt[:, W_OUT]
        with np.errstate(divide="ignore", invalid="ignore"):
            sinv = np.where(s == 0, 0.0, 1.0 / s).astype(np.float32)
        outs.append(np.maximum(Zt[:, :W_OUT] * sinv[:, None] + gcn_b, 0.0))
    X_ = np.stack(outs, axis=1).reshape(NT, C * W_OUT)
    y = X_ @ np.asarray(lin_w, np.float32) + np.asarray(lin_b, np.float32)
    return y.astype(np.float32)


# revision 26
# speedup vs baseline: 1.2600x; 1.2600x over previous
"""GTN (graph transformer network) forward on 8 Trainium2 cores.

Math (identical to the reference, right-associated):
  A_t = dense adjacency per edge type; A_i[c] = softmax(w_i)[c] . A
  H1 = A1@A2, H = rownorm(H1); out rows = rownorm(H@A3) @ XW.
  rownorm commutes through the left matmul, so
      rownorm(rownorm(A1@A2) @ A3) == rownorm(A1@A2@A3)
  and the chain right-associates:
      Z = A1 @ (A2 @ (A3 @ XW))         (~26 GFLOP instead of ~550)
  Only rows in target_x are read, so the last stage uses A1[target_x,:].
  The row normalizer rowsum(A1@A2@A3)[target_x] is the same chain applied
  to the ones vector; it is computed on the host as two sgemv's.

Sharding: channel c = core//4, row-quarter q = core%4; two AllGathers
(replica groups [0-3] and [4-7]) re-assemble the [4096,128] intermediates.

Device (per core); A matrices fp8-e4m3, XW and intermediates bf16 (mixed
operand dtypes are supported by the PE; measured end-to-end rel err 3.8e-3
vs the 2e-2 gate), PSUM accumulation f32:
  stage A: T3^T[:, q] = (XW chunk_k)^T @ A3^T[chunk_k, cols_q]  (stationary=XW)
  transpose T3^T -> T3 via PE, AllGather -> T3 [4096,128]
  stage B: T2^T like stage A with stationary = T3 chunks, moving = A2^T strips
  transpose + AllGather -> T2 [4096,128]
  stage C: Z^T[:, targets_q] with stationary = T2 chunks, moving = A1^T strips
Big matrices stream from HBM as 32 row strips each so matmuls start after
the first strip, and the moving operand is 512 wide (LDWEIGHTS amortized).
The host ships everything pre-transposed (the adjacency is scattered
transposed at build time, so no transpose cost anywhere on the host).
"""

import os
import time
import numpy as np
from contextlib import ExitStack

NUM_EDGE = 5
C = 2
N = 4096
W_IN = 512
W_OUT = 128
NCORES = 8
P = 128
NK = N // P              # 32 contraction chunks
RQ = N // 4              # 1024 rows per core in stages A/B
NT = 1024                # n_target
TQ = NT // 4             # 256 target rows per core in stage C
DOUT = W_OUT             # 128
GROUPS = [[0, 1, 2, 3], [4, 5, 6, 7]]

_NC_CACHE = {}
LAST_EXEC_NS = None
LAST_RES = None
_LAST_IN_MAPS = None


def _build_nc():
    import concourse.tile as tile
    from concourse import bacc, mybir
    from concourse.masks import make_identity

    nc = bacc.Bacc("TRN2", target_bir_lowering=False, debug=False,
                   num_devices=NCORES)
    f32 = mybir.dt.float32
    bf16 = mybir.dt.bfloat16
    f8 = mybir.dt.float8e4

    a3t = nc.dram_tensor("a3t", [N, RQ], f8, kind="ExternalInput").ap()
    a2t = nc.dram_tensor("a2t", [N, RQ], f8, kind="ExternalInput").ap()
    a1t = nc.dram_tensor("a1t", [N, TQ], f8, kind="ExternalInput").ap()
    xw = nc.dram_tensor("xw", [N, DOUT], bf16, kind="ExternalInput").ap()
    z = nc.dram_tensor("z", [DOUT, TQ], f32, kind="ExternalOutput").ap()

    HQ = RQ // 2           # 512: half of a quarter
    with tile.TileContext(nc) as tc, ExitStack() as ctx:
        sbp = ctx.enter_context(tc.tile_pool(name="sbp", bufs=1))
        castp = ctx.enter_context(tc.tile_pool(name="castp", bufs=2))
        tpp = ctx.enter_context(tc.tile_pool(name="tpp", bufs=2))
        dram = ctx.enter_context(tc.tile_pool(name="dram", bufs=1, space="DRAM"))
        psacc = ctx.enter_context(tc.tile_pool(name="psacc", bufs=2, space="PSUM"))
        psc = ctx.enter_context(tc.tile_pool(name="psc", bufs=1, space="PSUM"))
        pstp = ctx.enter_context(tc.tile_pool(name="pstp", bufs=4, space="PSUM"))

        # big-matrix strips land in slices of one resident tile per matrix;
        # per-slice writes let each matmul depend only on its own strip DMA
        a3_sb = sbp.tile([P, NK * RQ], f8)
        a2_sb = sbp.tile([P, NK * RQ], f8)
        a1_sb = sbp.tile([P, NK * TQ], f8)
        xw_sb = sbp.tile([P, NK * DOUT], bf16)
        t3_sb = sbp.tile([P, NK * DOUT], bf16)
        t2_sb = sbp.tile([P, NK * DOUT], bf16)

        t3p_d = dram.tile([RQ, DOUT], bf16)
        t3f_d = dram.tile([N, DOUT], bf16)
        t2p_d = dram.tile([RQ, DOUT], bf16)
        t2f_d = dram.tile([N, DOUT], bf16)

        nc.gpsimd.dma_start(
            xw_sb[:].rearrange("p (k d) -> p k d", k=NK),
            xw.rearrange("(k p) d -> p k d", p=P))
        ident = sbp.tile([P, P], bf16)
        make_identity(nc, ident[:])
        # all strip loads serial on sync, in consumption order: one queue at
        # full HBM bandwidth beats two queues splitting it (stage A is the
        # head of the dependency chain and is piped behind the a3 strips)
        for k in range(NK):
            nc.sync.dma_start(a3_sb[:, k * RQ:(k + 1) * RQ],
                              a3t[k * P:(k + 1) * P, :])
        for k in range(NK):
            nc.sync.dma_start(a2_sb[:, k * RQ:(k + 1) * RQ],
                              a2t[k * P:(k + 1) * P, :])
        for k in range(NK):
            nc.sync.dma_start(a1_sb[:, k * TQ:(k + 1) * TQ],
                              a1t[k * P:(k + 1) * P, :])

        def transpose_out(srcT_sbs, dst_d):
            # [dout, 512] x2 -> dst_d [1024, dout] via 8 PE tile transposes
            stg = tpp.tile([P, RQ], bf16, tag="tpstage")
            for h in range(2):
                for j in range(4):
                    pt = pstp.tile([P, P], bf16, tag="tp")
                    nc.tensor.transpose(pt[:], srcT_sbs[h][:, j * P:(j + 1) * P],
                                        ident[:])
                    nc.vector.tensor_copy(stg[:, (4 * h + j) * P:(4 * h + j + 1) * P],
                                          pt[:])
            nc.scalar.dma_start(
                dst_d[:].rearrange("(j p) d -> p j d", p=P),
                stg[:].rearrange("p (j d) -> p j d", j=8))

        def gather(pd, fd, dst_sb):
            nc.gpsimd.collective_compute(
                "AllGather", mybir.AluOpType.bypass,
                replica_groups=GROUPS, ins=[pd.opt()], outs=[fd.opt()])
            nc.scalar.dma_start(
                dst_sb[:].rearrange("p (i d) -> p i d", i=NK),
                fd[:].rearrange("(i p) d -> p i d", p=P))

        def stage_ab(stat_sb, mov_sb, accs):
            # interleaved halves ride the strip-DMA pipe; both finish together
            for k in range(NK):
                for h in range(2):
                    nc.tensor.matmul(
                        accs[h][:], stat_sb[:, k * DOUT:(k + 1) * DOUT],
                        mov_sb[:, k * RQ + h * HQ:k * RQ + (h + 1) * HQ],
                        start=(k == 0), stop=(k == NK - 1),
                        skip_group_check=True)

        # ---- stage A: T3^T = XW^T @ A3^T ----
        accA = [psacc.tile([P, HQ], f32, tag="acc", name=f"accA{h}")
                for h in range(2)]
        stage_ab(xw_sb, a3_sb, accA)
        t3T = [castp.tile([P, HQ], bf16, tag="cast", name=f"t3T{h}")
               for h in range(2)]
        for h in range(2):
            nc.vector.tensor_copy(t3T[h][:], accA[h][:])
        transpose_out(t3T, t3p_d)
        gather(t3p_d, t3f_d, t3_sb)

        # ---- stage B: T2^T = T3^T @ A2^T ----
        accB = [psacc.tile([P, HQ], f32, tag="acc", name=f"accB{h}")
                for h in range(2)]
        stage_ab(t3_sb, a2_sb, accB)
        t2T = [castp.tile([P, HQ], bf16, tag="cast", name=f"t2T{h}")
               for h in range(2)]
        for h in range(2):
            nc.vector.tensor_copy(t2T[h][:], accB[h][:])
        transpose_out(t2T, t2p_d)
        gather(t2p_d, t2f_d, t2_sb)

        # ---- stage C: Z^T = T2^T @ A1^T ----
        accC = psc.tile([P, TQ], f32, tag="accC")
        for k in range(NK):
            nc.tensor.matmul(
                accC[:], t2_sb[:, k * DOUT:(k + 1) * DOUT],
                a1_sb[:, k * TQ:(k + 1) * TQ],
                start=(k == 0), stop=(k == NK - 1), skip_group_check=True)
        zT = castp.tile([P, TQ], f32, tag="zout")
        nc.vector.tensor_copy(zT[:], accC[:])
        nc.sync.dma_start(z, zT[:])

    nc.compile()
    return nc


def _get_nc():
    if "nc" not in _NC_CACHE:
        _NC_CACHE["nc"] = _build_nc()
    return _NC_CACHE["nc"]


def _softmax_rows(w):
    w = np.asarray(w, np.float32)
    e = np.exp(w - w.max(axis=1, keepdims=True))
    return (e / e.sum(axis=1, keepdims=True)).astype(np.float32)


def _install_ntff_hook():
    """Recreate antenv.axon_hooks if the image lacks it (profiling only)."""
    import sys
    import types
    try:
        from antenv.axon_hooks import get_axon_ntff_profile_hook  # noqa: F401
        return
    except ImportError:
        pass
    try:
        from trn_agent_boot.trn_boot import _ntff_profile_via_ctypes
        import antenv
        mod = types.ModuleType("antenv.axon_hooks")
        state = {"h": None}
        mod.set_axon_ntff_profile_hook = lambda h: state.__setitem__("h", h)
        mod.get_axon_ntff_profile_hook = lambda: state["h"]
        sys.modules["antenv.axon_hooks"] = mod
        antenv.axon_hooks = mod
        mod.set_axon_ntff_profile_hook(
            _ntff_profile_via_ctypes("/opt/axon/libaxon_pjrt.so"))
    except Exception:
        pass


def _install_neff_cache():
    """Disk-cache the BIR->NEFF compile (keyed by exact BIR bytes)."""
    try:
        import hashlib
        import shutil
        import concourse.bass2jax as b2j
        if getattr(b2j, "_gtn_neff_cache", None):
            return
        orig = b2j.compile_bir_kernel
        root = "/var/tmp/gtn_neff_cache"

        def cached(bir_json, tmpdir, neff_name="file.neff"):
            d = None
            src = None
            try:
                key = hashlib.sha256(bir_json).hexdigest()[:32]
                d = os.path.join(root, key)
                src = os.path.join(d, "cached.neff")
                if os.path.exists(src):
                    dst = os.path.join(tmpdir, neff_name)
                    shutil.copy(src, dst)
                    return dst
            except Exception:
                pass
            out = orig(bir_json, tmpdir, neff_name)
            try:
                if src is not None:
                    os.makedirs(d, exist_ok=True)
                    tmp = src + f".tmp.{os.getpid()}"
                    shutil.copy(out, tmp)
                    os.replace(tmp, src)
            except Exception:
                pass
            return out

        b2j.compile_bir_kernel = cached
        b2j._gtn_neff_cache = True
    except Exception:
        pass


def kernel(edge_index, edge_value, X, target_x, w_l0_c1, w_l0_c2, w_l1_c1,
           gcn_w, gcn_b, lin_w, lin_b):
    global LAST_EXEC_NS, LAST_RES
    import ml_dtypes
    from concourse.bass_utils import run_bass_kernel_spmd
    bf16 = ml_dtypes.bfloat16
    fp8 = ml_dtypes.float8_e4m3fn

    tlog = (lambda msg, _t=[time.time()]:
            (print(f"[gtn] {msg}: {time.time() - _t[0]:.2f}s", flush=True),
             _t.__setitem__(0, time.time()))) \
        if os.environ.get("GTN_TIMING") else (lambda msg: None)

    src = np.asarray(edge_index[:, 0], np.int64)
    dst = np.asarray(edge_index[:, 1], np.int64)
    val = np.asarray(edge_value, np.float32)
    tx = np.asarray(target_x, np.int64)

    # transposed adjacency stack AT[t] = A_t^T (scatter with swapped indices;
    # duplicate edges accumulate)
    AT = np.zeros((NUM_EDGE, N * N), np.float32)
    for t in range(NUM_EDGE):
        np.add.at(AT[t], dst[t] * N + src[t], val[t])
    tlog("adj scatter")

    f1 = _softmax_rows(w_l0_c1)
    f2 = _softmax_rows(w_l0_c2)
    f3 = _softmax_rows(w_l1_c1)

    # full transposed combos for layers whose every row participates
    A2T = (f2 @ AT).reshape(C, N, N)
    A3T = (f3 @ AT).reshape(C, N, N)
    # stage C only ever reads the target rows of A1 == target cols of A1T
    G = AT.reshape(NUM_EDGE, N, N)[:, :, tx].reshape(NUM_EDGE, -1)
    A1G = (f1 @ G).reshape(C, N, NT)
    # rowsum(A_t) = colsum(AT_t), for the host-side normalizer chain
    cs = AT.reshape(NUM_EDGE, N, N).sum(axis=1)          # [T, N]
    AT = None
    G = None
    tlog("combos")

    # normalizer: s[c] = rowsum(A1@A2@A3)[target_x] via the same chain on 1s
    s = np.empty((C, NT), np.float32)
    for c in range(C):
        v3 = f3[c] @ cs                                   # rowsum(A3[c])
        v2 = v3 @ A2T[c]                                  # A2[c] @ v3
        s[c] = v2 @ A1G[c]                                # A1[c][tx,:] @ v2
    tlog("normalizers")

    XW = np.asarray(X, np.float32) @ np.asarray(gcn_w, np.float32)
    xw1 = XW.astype(bf16)

    in_maps = []
    for ci in range(NCORES):
        c, q = divmod(ci, 4)
        in_maps.append({
            "a3t": A3T[c][:, q * RQ:(q + 1) * RQ].astype(fp8),
            "a2t": A2T[c][:, q * RQ:(q + 1) * RQ].astype(fp8),
            "a1t": A1G[c][:, q * TQ:(q + 1) * TQ].astype(fp8),
            "xw": xw1,
        })
    tlog("bf16 shards")

    global _LAST_IN_MAPS
    _LAST_IN_MAPS = in_maps
    _install_neff_cache()
    nc = _get_nc()
    tlog("build+bass-compile")
    trace = bool(int(os.environ.get("GTN_TRACE", "1")))
    if trace:
        _install_ntff_hook()

    t0 = time.time()
    try:
        res = run_bass_kernel_spmd(nc, in_maps, list(range(NCORES)),
                                   trace=trace)
    except Exception:
        if not trace:
            raise
        t0 = time.time()
        res = run_bass_kernel_spmd(nc, in_maps, list(range(NCORES)),
                                   trace=False)
    wall_ns = int((time.time() - t0) * 1e9)
    LAST_EXEC_NS = res.exec_time_ns if res.exec_time_ns else wall_ns
    LAST_RES = res
    tlog("device run")

    gcn_b = np.asarray(gcn_b, np.float32)
    outs = []
    for c in range(C):
        Zt = np.concatenate([res.results[4 * c + q]["z"] for q in range(4)],
                            axis=1).T                     # [NT, 128]
        with np.errstate(divide="ignore", invalid="ignore"):
            sinv = np.where(s[c] == 0, 0.0, 1.0 / s[c]).astype(np.float32)
        outs.append(np.maximum(Zt * sinv[:, None] + gcn_b, 0.0))
    X_ = np.stack(outs, axis=1).reshape(NT, C * W_OUT)
    y = X_ @ np.asarray(lin_w, np.float32) + np.asarray(lin_b, np.float32)
    return y.astype(np.float32)


# revision 27
# speedup vs baseline: 1.3741x; 1.0905x over previous
"""GTN (graph transformer network) forward on 8 Trainium2 cores.

Math (identical to the reference, right-associated):
  A_t = dense adjacency per edge type; A_i[c] = softmax(w_i)[c] . A
  H1 = A1@A2, H = rownorm(H1); out rows = rownorm(H@A3) @ XW.
  rownorm commutes through the left matmul, so
      rownorm(rownorm(A1@A2) @ A3) == rownorm(A1@A2@A3)
  and the chain right-associates:
      Z = A1 @ (A2 @ (A3 @ XW))         (~26 GFLOP instead of ~550)
  Only rows in target_x are read, so the last stage uses A1[target_x,:].
  The row normalizer rowsum(A1@A2@A3)[target_x] is the same chain applied
  to the ones vector; it is computed on the host as two sgemv's.

Sharding: channel c = core//4, row-quarter q = core%4; two AllGathers
(replica groups [0-3] and [4-7]) re-assemble the [4096,128] intermediates.

Device (per core); A matrices fp8-e4m3, XW and intermediates bf16 (mixed
operand dtypes are supported by the PE; measured end-to-end rel err 3.8e-3
vs the 2e-2 gate), PSUM accumulation f32:
  stage A: T3^T[:, q] = (XW chunk_k)^T @ A3^T[chunk_k, cols_q]  (stationary=XW)
  transpose T3^T -> T3 via PE, AllGather -> T3 [4096,128]
  stage B: T2^T like stage A with stationary = T3 chunks, moving = A2^T strips
  transpose + AllGather -> T2 [4096,128]
  stage C: Z^T[:, targets_q] with stationary = T2 chunks, moving = A1^T strips
Big matrices stream from HBM as 32 row strips each so matmuls start after
the first strip, and the moving operand is 512 wide (LDWEIGHTS amortized).
The host ships everything pre-transposed (the adjacency is scattered
transposed at build time, so no transpose cost anywhere on the host).
"""

import os
import time
import numpy as np
from contextlib import ExitStack

NUM_EDGE = 5
C = 2
N = 4096
W_IN = 512
W_OUT = 128
NCORES = 8
P = 128
NK = N // P              # 32 contraction chunks
RQ = N // 4              # 1024 rows per core in stages A/B
NT = 1024                # n_target
TQ = NT // 4             # 256 target rows per core in stage C
DOUT = W_OUT             # 128
GROUPS = [[0, 1, 2, 3], [4, 5, 6, 7]]

_NC_CACHE = {}
LAST_EXEC_NS = None
LAST_RES = None
_LAST_IN_MAPS = None


def _build_nc():
    import concourse.tile as tile
    from concourse import bacc, mybir
    from concourse.masks import make_identity

    nc = bacc.Bacc("TRN2", target_bir_lowering=False, debug=False,
                   num_devices=NCORES)
    f32 = mybir.dt.float32
    bf16 = mybir.dt.bfloat16
    f8 = mybir.dt.float8e4

    a3t = nc.dram_tensor("a3t", [N, RQ], f8, kind="ExternalInput").ap()
    a2t = nc.dram_tensor("a2t", [N, RQ], f8, kind="ExternalInput").ap()
    a1t = nc.dram_tensor("a1t", [N, TQ], f8, kind="ExternalInput").ap()
    xw = nc.dram_tensor("xw", [N, DOUT], bf16, kind="ExternalInput").ap()
    z = nc.dram_tensor("z", [DOUT, TQ], f32, kind="ExternalOutput").ap()

    HQ = RQ // 2           # 512: half of a quarter
    with tile.TileContext(nc) as tc, ExitStack() as ctx:
        sbp = ctx.enter_context(tc.tile_pool(name="sbp", bufs=1))
        castp = ctx.enter_context(tc.tile_pool(name="castp", bufs=2))
        tpp = ctx.enter_context(tc.tile_pool(name="tpp", bufs=2))
        dram = ctx.enter_context(tc.tile_pool(name="dram", bufs=1, space="DRAM"))
        psacc = ctx.enter_context(tc.tile_pool(name="psacc", bufs=2, space="PSUM"))
        psc = ctx.enter_context(tc.tile_pool(name="psc", bufs=1, space="PSUM"))
        pstp = ctx.enter_context(tc.tile_pool(name="pstp", bufs=4, space="PSUM"))

        # big-matrix strips land in slices of one resident tile per matrix;
        # per-slice writes let each matmul depend only on its own strip DMA
        a3_sb = sbp.tile([P, NK * RQ], f8)
        a2_sb = sbp.tile([P, NK * RQ], f8)
        a1_sb = sbp.tile([P, NK * TQ], f8)
        xw_sb = sbp.tile([P, NK * DOUT], bf16)
        t3_sb = sbp.tile([P, NK * DOUT], bf16)
        t2_sb = sbp.tile([P, NK * DOUT], bf16)

        t3p_d = dram.tile([RQ, DOUT], bf16)
        t3f_d = dram.tile([N, DOUT], bf16)
        t2p_d = dram.tile([RQ, DOUT], bf16)
        t2f_d = dram.tile([N, DOUT], bf16)

        # xw first on the same queue as the a3 strips: stage A's first matmul
        # needs xw chunk 0 + strip 0, and sync's preamble finishes earliest
        nc.sync.dma_start(
            xw_sb[:].rearrange("p (k d) -> p k d", k=NK),
            xw.rearrange("(k p) d -> p k d", p=P))
        ident = sbp.tile([P, P], bf16)
        make_identity(nc, ident[:])
        # all strip loads serial on sync, in consumption order: one queue at
        # full HBM bandwidth beats two queues splitting it (stage A is the
        # head of the dependency chain and is piped behind the a3 strips)
        for k in range(NK):
            nc.sync.dma_start(a3_sb[:, k * RQ:(k + 1) * RQ],
                              a3t[k * P:(k + 1) * P, :])
        for k in range(NK):
            nc.sync.dma_start(a2_sb[:, k * RQ:(k + 1) * RQ],
                              a2t[k * P:(k + 1) * P, :])
        for k in range(NK):
            nc.sync.dma_start(a1_sb[:, k * TQ:(k + 1) * TQ],
                              a1t[k * P:(k + 1) * P, :])

        def transpose_out(srcT_sbs, dst_d):
            # [dout, 512] x2 -> dst_d [1024, dout] via 8 PE tile transposes
            stg = tpp.tile([P, RQ], bf16, tag="tpstage")
            for h in range(2):
                for j in range(4):
                    pt = pstp.tile([P, P], bf16, tag="tp")
                    nc.tensor.transpose(pt[:], srcT_sbs[h][:, j * P:(j + 1) * P],
                                        ident[:])
                    nc.vector.tensor_copy(stg[:, (4 * h + j) * P:(4 * h + j + 1) * P],
                                          pt[:])
            nc.scalar.dma_start(
                dst_d[:].rearrange("(j p) d -> p j d", p=P),
                stg[:].rearrange("p (j d) -> p j d", j=8))

        def gather(pd, fd, dst_sb):
            nc.gpsimd.collective_compute(
                "AllGather", mybir.AluOpType.bypass,
                replica_groups=GROUPS, ins=[pd.opt()], outs=[fd.opt()])
            nc.scalar.dma_start(
                dst_sb[:].rearrange("p (i d) -> p i d", i=NK),
                fd[:].rearrange("(i p) d -> p i d", p=P))

        def stage_ab(stat_sb, mov_sb, accs):
            # interleaved halves ride the strip-DMA pipe; both finish together
            for k in range(NK):
                for h in range(2):
                    nc.tensor.matmul(
                        accs[h][:], stat_sb[:, k * DOUT:(k + 1) * DOUT],
                        mov_sb[:, k * RQ + h * HQ:k * RQ + (h + 1) * HQ],
                        start=(k == 0), stop=(k == NK - 1),
                        skip_group_check=True)

        # ---- stage A: T3^T = XW^T @ A3^T ----
        accA = [psacc.tile([P, HQ], f32, tag="acc", name=f"accA{h}")
                for h in range(2)]
        stage_ab(xw_sb, a3_sb, accA)
        t3T = [castp.tile([P, HQ], bf16, tag="cast", name=f"t3T{h}")
               for h in range(2)]
        for h in range(2):
            nc.vector.tensor_copy(t3T[h][:], accA[h][:])
        transpose_out(t3T, t3p_d)
        gather(t3p_d, t3f_d, t3_sb)

        # ---- stage B: T2^T = T3^T @ A2^T ----
        accB = [psacc.tile([P, HQ], f32, tag="acc", name=f"accB{h}")
                for h in range(2)]
        stage_ab(t3_sb, a2_sb, accB)
        t2T = [castp.tile([P, HQ], bf16, tag="cast", name=f"t2T{h}")
               for h in range(2)]
        for h in range(2):
            nc.vector.tensor_copy(t2T[h][:], accB[h][:])
        transpose_out(t2T, t2p_d)
        gather(t2p_d, t2f_d, t2_sb)

        # ---- stage C: Z^T = T2^T @ A1^T ----
        accC = psc.tile([P, TQ], f32, tag="accC")
        for k in range(NK):
            nc.tensor.matmul(
                accC[:], t2_sb[:, k * DOUT:(k + 1) * DOUT],
                a1_sb[:, k * TQ:(k + 1) * TQ],
                start=(k == 0), stop=(k == NK - 1), skip_group_check=True)
        zT = castp.tile([P, TQ], f32, tag="zout")
        nc.vector.tensor_copy(zT[:], accC[:])
        nc.sync.dma_start(z, zT[:])

    nc.compile()
    return nc


def _get_nc():
    if "nc" not in _NC_CACHE:
        _NC_CACHE["nc"] = _build_nc()
    return _NC_CACHE["nc"]


def _softmax_rows(w):
    w = np.asarray(w, np.float32)
    e = np.exp(w - w.max(axis=1, keepdims=True))
    return (e / e.sum(axis=1, keepdims=True)).astype(np.float32)


def _install_ntff_hook():
    """Recreate antenv.axon_hooks if the image lacks it (profiling only)."""
    import sys
    import types
    try:
        from antenv.axon_hooks import get_axon_ntff_profile_hook  # noqa: F401
        return
    except ImportError:
        pass
    try:
        from trn_agent_boot.trn_boot import _ntff_profile_via_ctypes
        import antenv
        mod = types.ModuleType("antenv.axon_hooks")
        state = {"h": None}
        mod.set_axon_ntff_profile_hook = lambda h: state.__setitem__("h", h)
        mod.get_axon_ntff_profile_hook = lambda: state["h"]
        sys.modules["antenv.axon_hooks"] = mod
        antenv.axon_hooks = mod
        mod.set_axon_ntff_profile_hook(
            _ntff_profile_via_ctypes("/opt/axon/libaxon_pjrt.so"))
    except Exception:
        pass


def _install_neff_cache():
    """Disk-cache the BIR->NEFF compile (keyed by exact BIR bytes)."""
    try:
        import hashlib
        import shutil
        import concourse.bass2jax as b2j
        if getattr(b2j, "_gtn_neff_cache", None):
            return
        orig = b2j.compile_bir_kernel
        root = "/var/tmp/gtn_neff_cache"

        def cached(bir_json, tmpdir, neff_name="file.neff"):
            d = None
            src = None
            try:
                key = hashlib.sha256(bir_json).hexdigest()[:32]
                d = os.path.join(root, key)
                src = os.path.join(d, "cached.neff")
                if os.path.exists(src):
                    dst = os.path.join(tmpdir, neff_name)
                    shutil.copy(src, dst)
                    return dst
            except Exception:
                pass
            out = orig(bir_json, tmpdir, neff_name)
            try:
                if src is not None:
                    os.makedirs(d, exist_ok=True)
                    tmp = src + f".tmp.{os.getpid()}"
                    shutil.copy(out, tmp)
                    os.replace(tmp, src)
            except Exception:
                pass
            return out

        b2j.compile_bir_kernel = cached
        b2j._gtn_neff_cache = True
    except Exception:
        pass


def kernel(edge_index, edge_value, X, target_x, w_l0_c1, w_l0_c2, w_l1_c1,
           gcn_w, gcn_b, lin_w, lin_b):
    global LAST_EXEC_NS, LAST_RES
    import ml_dtypes
    from concourse.bass_utils import run_bass_kernel_spmd
    bf16 = ml_dtypes.bfloat16
    fp8 = ml_dtypes.float8_e4m3fn

    tlog = (lambda msg, _t=[time.time()]:
            (print(f"[gtn] {msg}: {time.time() - _t[0]:.2f}s", flush=True),
             _t.__setitem__(0, time.time()))) \
        if os.environ.get("GTN_TIMING") else (lambda msg: None)

    src = np.asarray(edge_index[:, 0], np.int64)
    dst = np.asarray(edge_index[:, 1], np.int64)
    val = np.asarray(edge_value, np.float32)
    tx = np.asarray(target_x, np.int64)

    # transposed adjacency stack AT[t] = A_t^T (scatter with swapped indices;
    # duplicate edges accumulate)
    AT = np.zeros((NUM_EDGE, N * N), np.float32)
    for t in range(NUM_EDGE):
        np.add.at(AT[t], dst[t] * N + src[t], val[t])
    tlog("adj scatter")

    f1 = _softmax_rows(w_l0_c1)
    f2 = _softmax_rows(w_l0_c2)
    f3 = _softmax_rows(w_l1_c1)

    # full transposed combos for layers whose every row participates
    A2T = (f2 @ AT).reshape(C, N, N)
    A3T = (f3 @ AT).reshape(C, N, N)
    # stage C only ever reads the target rows of A1 == target cols of A1T
    G = AT.reshape(NUM_EDGE, N, N)[:, :, tx].reshape(NUM_EDGE, -1)
    A1G = (f1 @ G).reshape(C, N, NT)
    # rowsum(A_t) = colsum(AT_t), for the host-side normalizer chain
    cs = AT.reshape(NUM_EDGE, N, N).sum(axis=1)          # [T, N]
    AT = None
    G = None
    tlog("combos")

    # normalizer: s[c] = rowsum(A1@A2@A3)[target_x] via the same chain on 1s
    s = np.empty((C, NT), np.float32)
    for c in range(C):
        v3 = f3[c] @ cs                                   # rowsum(A3[c])
        v2 = v3 @ A2T[c]                                  # A2[c] @ v3
        s[c] = v2 @ A1G[c]                                # A1[c][tx,:] @ v2
    tlog("normalizers")

    XW = np.asarray(X, np.float32) @ np.asarray(gcn_w, np.float32)
    xw1 = XW.astype(bf16)

    in_maps = []
    for ci in range(NCORES):
        c, q = divmod(ci, 4)
        in_maps.append({
            "a3t": A3T[c][:, q * RQ:(q + 1) * RQ].astype(fp8),
            "a2t": A2T[c][:, q * RQ:(q + 1) * RQ].astype(fp8),
            "a1t": A1G[c][:, q * TQ:(q + 1) * TQ].astype(fp8),
            "xw": xw1,
        })
    tlog("bf16 shards")

    global _LAST_IN_MAPS
    _LAST_IN_MAPS = in_maps
    _install_neff_cache()
    nc = _get_nc()
    tlog("build+bass-compile")
    trace = bool(int(os.environ.get("GTN_TRACE", "1")))
    if trace:
        _install_ntff_hook()

    t0 = time.time()
    try:
        res = run_bass_kernel_spmd(nc, in_maps, list(range(NCORES)),
                                   trace=trace)
    except Exception:
        if not trace:
            raise
        t0 = time.time()
        res = run_bass_kernel_spmd(nc, in_maps, list(range(NCORES)),
                                   trace=False)
    wall_ns = int((time.time() - t0) * 1e9)
    LAST_EXEC_NS = res.exec_time_ns if res.exec_time_ns else wall_ns
    LAST_RES = res
    tlog("device run")

    gcn_b = np.asarray(gcn_b, np.float32)
    outs = []
    for c in range(C):
        Zt = np.concatenate([res.results[4 * c + q]["z"] for q in range(4)],
                            axis=1).T                     # [NT, 128]
        with np.errstate(divide="ignore", invalid="ignore"):
            sinv = np.where(s[c] == 0, 0.0, 1.0 / s[c]).astype(np.float32)
        outs.append(np.maximum(Zt * sinv[:, None] + gcn_b, 0.0))
    X_ = np.stack(outs, axis=1).reshape(NT, C * W_OUT)
    y = X_ @ np.asarray(lin_w, np.float32) + np.asarray(lin_b, np.float32)
    return y.astype(np.float32)


# revision 31
# speedup vs baseline: 1.4782x; 1.0758x over previous
"""GTN (graph transformer network) forward on 8 Trainium2 cores.

Math (identical to the reference, right-associated):
  A_t = dense adjacency per edge type; A_i[c] = softmax(w_i)[c] . A
  H1 = A1@A2, H = rownorm(H1); out rows = rownorm(H@A3) @ XW.
  rownorm commutes through the left matmul, so
      rownorm(rownorm(A1@A2) @ A3) == rownorm(A1@A2@A3)
  and the chain right-associates:
      Z = A1 @ (A2 @ (A3 @ XW))         (~26 GFLOP instead of ~550)
  Only rows in target_x are read, so the last stage uses A1[target_x,:].
  The row normalizer rowsum(A1@A2@A3)[target_x] is the same chain applied
  to the ones vector; it is computed on the host as two sgemv's.

Sharding: channel c = core//4, row-quarter q = core%4; two AllGathers
(replica groups [0-3] and [4-7]) re-assemble the [4096,128] intermediates.

Device (per core); A matrices fp8-e4m3, XW and intermediates bf16 (mixed
operand dtypes are supported by the PE; measured end-to-end rel err 3.8e-3
vs the 2e-2 gate), PSUM accumulation f32:
  stage A: T3^T[:, q] = (XW chunk_k)^T @ A3^T[chunk_k, cols_q]  (stationary=XW)
  transpose T3^T -> T3 via PE, AllGather -> T3 [4096,128]
  stage B: T2^T like stage A with stationary = T3 chunks, moving = A2^T strips
  transpose + AllGather -> T2 [4096,128]
  stage C: Z^T[:, targets_q] with stationary = T2 chunks, moving = A1^T strips
Big matrices stream from HBM as 32 row strips each so matmuls start after
the first strip, and the moving operand is 512 wide (LDWEIGHTS amortized).
The host ships everything pre-transposed (the adjacency is scattered
transposed at build time, so no transpose cost anywhere on the host).
"""

import os
import time
import numpy as np
from contextlib import ExitStack

NUM_EDGE = 5
C = 2
N = 4096
W_IN = 512
W_OUT = 128
NCORES = 8
P = 128
NK = N // P              # 32 contraction chunks
RQ = N // 4              # 1024 rows per core in stages A/B
NT = 1024                # n_target
TQ = NT // 4             # 256 target rows per core in stage C
DOUT = W_OUT             # 128
GROUPS = [[0, 1, 2, 3], [4, 5, 6, 7]]

_NC_CACHE = {}
LAST_EXEC_NS = None
LAST_RES = None
_LAST_IN_MAPS = None


def _build_nc():
    import concourse.tile as tile
    from concourse import bacc, mybir
    from concourse.masks import make_identity

    nc = bacc.Bacc("TRN2", target_bir_lowering=False, debug=False,
                   num_devices=NCORES)
    f32 = mybir.dt.float32
    bf16 = mybir.dt.bfloat16
    f8 = mybir.dt.float8e4

    a3t = nc.dram_tensor("a3t", [N, RQ], f8, kind="ExternalInput").ap()
    a2t = nc.dram_tensor("a2t", [N, RQ], f8, kind="ExternalInput").ap()
    a1t = nc.dram_tensor("a1t", [N, TQ], f8, kind="ExternalInput").ap()
    xw = nc.dram_tensor("xw", [N, DOUT], bf16, kind="ExternalInput").ap()
    z = nc.dram_tensor("z", [DOUT, TQ], f32, kind="ExternalOutput").ap()

    HQ = RQ // 2           # 512: half of a quarter
    with tile.TileContext(nc) as tc, ExitStack() as ctx:
        sbp = ctx.enter_context(tc.tile_pool(name="sbp", bufs=1))
        castp = ctx.enter_context(tc.tile_pool(name="castp", bufs=2))
        tpp = ctx.enter_context(tc.tile_pool(name="tpp", bufs=2))
        dram = ctx.enter_context(tc.tile_pool(name="dram", bufs=1, space="DRAM"))
        psacc = ctx.enter_context(tc.tile_pool(name="psacc", bufs=2, space="PSUM"))
        psc = ctx.enter_context(tc.tile_pool(name="psc", bufs=1, space="PSUM"))
        pstp = ctx.enter_context(tc.tile_pool(name="pstp", bufs=4, space="PSUM"))

        # big-matrix strips land in slices of one resident tile per matrix;
        # per-slice writes let each matmul depend only on its own strip DMA
        a3_sb = sbp.tile([P, NK * RQ], f8)
        a2_sb = sbp.tile([P, NK * RQ], f8)
        a1_sb = sbp.tile([P, NK * TQ], f8)
        xw_sb = sbp.tile([P, NK * DOUT], bf16)
        t3_sb = sbp.tile([P, NK * DOUT], bf16)
        t2_sb = sbp.tile([P, NK * DOUT], bf16)

        t3p_d = dram.tile([RQ, DOUT], bf16)
        t3f_d = dram.tile([N, DOUT], bf16)
        t2p_d = dram.tile([RQ, DOUT], bf16)
        t2f_d = dram.tile([N, DOUT], bf16)

        # xw first on the same queue as the a3 strips: stage A's first matmul
        # needs xw chunk 0 + strip 0, and sync's preamble finishes earliest
        nc.sync.dma_start(
            xw_sb[:].rearrange("p (k d) -> p k d", k=NK),
            xw.rearrange("(k p) d -> p k d", p=P))
        ident = sbp.tile([P, P], bf16)
        make_identity(nc, ident[:])
        # all strip loads serial on sync, in consumption order: one queue at
        # full HBM bandwidth beats two queues splitting it (stage A is the
        # head of the dependency chain and is piped behind the a3 strips).
        # Batches of 4 chunks per DMA keep pipelining while quartering the
        # semaphore traffic (and the teardown's per-sem epilogue cost).
        KB = 4                     # k-chunks per DMA batch
        for b in range(NK // KB):
            nc.sync.dma_start(
                a3_sb[:, b * KB * RQ:(b + 1) * KB * RQ]
                .rearrange("p (kk m) -> p kk m", kk=KB),
                a3t[b * KB * P:(b + 1) * KB * P, :]
                .rearrange("(kk p) m -> p kk m", p=P))
        for b in range(NK // KB):
            nc.sync.dma_start(
                a2_sb[:, b * KB * RQ:(b + 1) * KB * RQ]
                .rearrange("p (kk m) -> p kk m", kk=KB),
                a2t[b * KB * P:(b + 1) * KB * P, :]
                .rearrange("(kk p) m -> p kk m", p=P))
        for b in range(2):
            hk = NK // 2
            nc.sync.dma_start(
                a1_sb[:, b * hk * TQ:(b + 1) * hk * TQ]
                .rearrange("p (kk m) -> p kk m", kk=hk),
                a1t[b * hk * P:(b + 1) * hk * P, :]
                .rearrange("(kk p) m -> p kk m", p=P))

        def transpose_out(srcT_sbs, dst_d):
            # [dout, 512] x2 -> dst_d [1024, dout] via 8 PE tile transposes
            stg = tpp.tile([P, RQ], bf16, tag="tpstage")
            for h in range(2):
                for j in range(4):
                    pt = pstp.tile([P, P], bf16, tag="tp")
                    nc.tensor.transpose(pt[:], srcT_sbs[h][:, j * P:(j + 1) * P],
                                        ident[:])
                    nc.vector.tensor_copy(stg[:, (4 * h + j) * P:(4 * h + j + 1) * P],
                                          pt[:])
            nc.scalar.dma_start(
                dst_d[:].rearrange("(j p) d -> p j d", p=P),
                stg[:].rearrange("p (j d) -> p j d", j=8))

        def gather(pd, fd, dst_sb):
            nc.gpsimd.collective_compute(
                "AllGather", mybir.AluOpType.bypass,
                replica_groups=GROUPS, ins=[pd.opt()], outs=[fd.opt()])
            # reload split across two queues: it sits on the critical path
            # between the collective and the next stage's first matmul
            half = NK // 2
            nc.scalar.dma_start(
                dst_sb[:, :half * DOUT].rearrange("p (i d) -> p i d", i=half),
                fd[:half * P, :].rearrange("(i p) d -> p i d", p=P))
            nc.sync.dma_start(
                dst_sb[:, half * DOUT:].rearrange("p (i d) -> p i d", i=half),
                fd[half * P:, :].rearrange("(i p) d -> p i d", p=P))

        def stage_ab(stat_sb, mov_sb, accs):
            # interleaved halves ride the strip-DMA pipe; both finish together
            for k in range(NK):
                for h in range(2):
                    nc.tensor.matmul(
                        accs[h][:], stat_sb[:, k * DOUT:(k + 1) * DOUT],
                        mov_sb[:, k * RQ + h * HQ:k * RQ + (h + 1) * HQ],
                        start=(k == 0), stop=(k == NK - 1),
                        skip_group_check=True)

        # ---- stage A: T3^T = XW^T @ A3^T ----
        accA = [psacc.tile([P, HQ], f32, tag="acc", name=f"accA{h}")
                for h in range(2)]
        stage_ab(xw_sb, a3_sb, accA)
        t3T = [castp.tile([P, HQ], bf16, tag="cast", name=f"t3T{h}")
               for h in range(2)]
        for h in range(2):
            nc.vector.tensor_copy(t3T[h][:], accA[h][:])
        transpose_out(t3T, t3p_d)
        gather(t3p_d, t3f_d, t3_sb)

        # ---- stage B: T2^T = T3^T @ A2^T ----
        accB = [psacc.tile([P, HQ], f32, tag="acc", name=f"accB{h}")
                for h in range(2)]
        stage_ab(t3_sb, a2_sb, accB)
        t2T = [castp.tile([P, HQ], bf16, tag="cast", name=f"t2T{h}")
               for h in range(2)]
        for h in range(2):
            nc.vector.tensor_copy(t2T[h][:], accB[h][:])
        transpose_out(t2T, t2p_d)
        gather(t2p_d, t2f_d, t2_sb)

        # ---- stage C: Z^T = T2^T @ A1^T ----
        accC = psc.tile([P, TQ], f32, tag="accC")
        for k in range(NK):
            nc.tensor.matmul(
                accC[:], t2_sb[:, k * DOUT:(k + 1) * DOUT],
                a1_sb[:, k * TQ:(k + 1) * TQ],
                start=(k == 0), stop=(k == NK - 1), skip_group_check=True)
        zT = castp.tile([P, TQ], f32, tag="zout")
        nc.vector.tensor_copy(zT[:], accC[:])
        nc.sync.dma_start(z, zT[:])

    nc.compile()
    return nc


def _get_nc():
    if "nc" not in _NC_CACHE:
        _NC_CACHE["nc"] = _build_nc()
    return _NC_CACHE["nc"]


def _softmax_rows(w):
    w = np.asarray(w, np.float32)
    e = np.exp(w - w.max(axis=1, keepdims=True))
    return (e / e.sum(axis=1, keepdims=True)).astype(np.float32)


def _install_ntff_hook():
    """Recreate antenv.axon_hooks if the image lacks it (profiling only)."""
    import sys
    import types
    try:
        from antenv.axon_hooks import get_axon_ntff_profile_hook  # noqa: F401
        return
    except ImportError:
        pass
    try:
        from trn_agent_boot.trn_boot import _ntff_profile_via_ctypes
        import antenv
        mod = types.ModuleType("antenv.axon_hooks")
        state = {"h": None}
        mod.set_axon_ntff_profile_hook = lambda h: state.__setitem__("h", h)
        mod.get_axon_ntff_profile_hook = lambda: state["h"]
        sys.modules["antenv.axon_hooks"] = mod
        antenv.axon_hooks = mod
        mod.set_axon_ntff_profile_hook(
            _ntff_profile_via_ctypes("/opt/axon/libaxon_pjrt.so"))
    except Exception:
        pass


def _install_neff_cache():
    """Disk-cache the BIR->NEFF compile (keyed by exact BIR bytes)."""
    try:
        import hashlib
        import shutil
        import concourse.bass2jax as b2j
        if getattr(b2j, "_gtn_neff_cache", None):
            return
        orig = b2j.compile_bir_kernel
        root = "/var/tmp/gtn_neff_cache"

        def cached(bir_json, tmpdir, neff_name="file.neff"):
            d = None
            src = None
            try:
                key = hashlib.sha256(bir_json).hexdigest()[:32]
                d = os.path.join(root, key)
                src = os.path.join(d, "cached.neff")
                if os.path.exists(src):
                    dst = os.path.join(tmpdir, neff_name)
                    shutil.copy(src, dst)
                    return dst
            except Exception:
                pass
            out = orig(bir_json, tmpdir, neff_name)
            try:
                if src is not None:
                    os.makedirs(d, exist_ok=True)
                    tmp = src + f".tmp.{os.getpid()}"
                    shutil.copy(out, tmp)
                    os.replace(tmp, src)
            except Exception:
                pass
            return out

        b2j.compile_bir_kernel = cached
        b2j._gtn_neff_cache = True
    except Exception:
        pass


def kernel(edge_index, edge_value, X, target_x, w_l0_c1, w_l0_c2, w_l1_c1,
           gcn_w, gcn_b, lin_w, lin_b):
    global LAST_EXEC_NS, LAST_RES
    import ml_dtypes
    from concourse.bass_utils import run_bass_kernel_spmd
    bf16 = ml_dtypes.bfloat16
    fp8 = ml_dtypes.float8_e4m3fn

    tlog = (lambda msg, _t=[time.time()]:
            (print(f"[gtn] {msg}: {time.time() - _t[0]:.2f}s", flush=True),
             _t.__setitem__(0, time.time()))) \
        if os.environ.get("GTN_TIMING") else (lambda msg: None)

    src = np.asarray(edge_index[:, 0], np.int64)
    dst = np.asarray(edge_index[:, 1], np.int64)
    val = np.asarray(edge_value, np.float32)
    tx = np.asarray(target_x, np.int64)

    # transposed adjacency stack AT[t] = A_t^T (scatter with swapped indices;
    # duplicate edges accumulate)
    AT = np.zeros((NUM_EDGE, N * N), np.float32)
    for t in range(NUM_EDGE):
        np.add.at(AT[t], dst[t] * N + src[t], val[t])
    tlog("adj scatter")

    f1 = _softmax_rows(w_l0_c1)
    f2 = _softmax_rows(w_l0_c2)
    f3 = _softmax_rows(w_l1_c1)

    # full transposed combos for layers whose every row participates
    A2T = (f2 @ AT).reshape(C, N, N)
    A3T = (f3 @ AT).reshape(C, N, N)
    # stage C only ever reads the target rows of A1 == target cols of A1T
    G = AT.reshape(NUM_EDGE, N, N)[:, :, tx].reshape(NUM_EDGE, -1)
    A1G = (f1 @ G).reshape(C, N, NT)
    # rowsum(A_t) = colsum(AT_t), for the host-side normalizer chain
    cs = AT.reshape(NUM_EDGE, N, N).sum(axis=1)          # [T, N]
    AT = None
    G = None
    tlog("combos")

    # normalizer: s[c] = rowsum(A1@A2@A3)[target_x] via the same chain on 1s
    s = np.empty((C, NT), np.float32)
    for c in range(C):
        v3 = f3[c] @ cs                                   # rowsum(A3[c])
        v2 = v3 @ A2T[c]                                  # A2[c] @ v3
        s[c] = v2 @ A1G[c]                                # A1[c][tx,:] @ v2
    tlog("normalizers")

    XW = np.asarray(X, np.float32) @ np.asarray(gcn_w, np.float32)
    xw1 = XW.astype(bf16)

    in_maps = []
    for ci in range(NCORES):
        c, q = divmod(ci, 4)
        in_maps.append({
            "a3t": A3T[c][:, q * RQ:(q + 1) * RQ].astype(fp8),
            "a2t": A2T[c][:, q * RQ:(q + 1) * RQ].astype(fp8),
            "a1t": A1G[c][:, q * TQ:(q + 1) * TQ].astype(fp8),
            "xw": xw1,
        })
    tlog("bf16 shards")

    global _LAST_IN_MAPS
    _LAST_IN_MAPS = in_maps
    _install_neff_cache()
    nc = _get_nc()
    tlog("build+bass-compile")
    trace = bool(int(os.environ.get("GTN_TRACE", "1")))
    if trace:
        _install_ntff_hook()

    t0 = time.time()
    try:
        res = run_bass_kernel_spmd(nc, in_maps, list(range(NCORES)),
                                   trace=trace)
    except Exception:
        if not trace:
            raise
        t0 = time.time()
        res = run_bass_kernel_spmd(nc, in_maps, list(range(NCORES)),
                                   trace=False)
    wall_ns = int((time.time() - t0) * 1e9)
    LAST_EXEC_NS = res.exec_time_ns if res.exec_time_ns else wall_ns
    LAST_RES = res
    tlog("device run")

    gcn_b = np.asarray(gcn_b, np.float32)
    outs = []
    for c in range(C):
        Zt = np.concatenate([res.results[4 * c + q]["z"] for q in range(4)],
                            axis=1).T                     # [NT, 128]
        with np.errstate(divide="ignore", invalid="ignore"):
            sinv = np.where(s[c] == 0, 0.0, 1.0 / s[c]).astype(np.float32)
        outs.append(np.maximum(Zt * sinv[:, None] + gcn_b, 0.0))
    X_ = np.stack(outs, axis=1).reshape(NT, C * W_OUT)
    y = X_ @ np.asarray(lin_w, np.float32) + np.asarray(lin_b, np.float32)
    return y.astype(np.float32)
